# revision 3
# baseline (speedup 1.0000x reference)
"""Trainium2 Bass kernel for nn_ContKDLoss (NT-Xent contrastive + KD softmax-KL + MSE).

Strategy (8 NeuronCores, data parallel over batch):
  - core c owns 256 target rows + 256 surrogate rows (512 "local rows").
  - phase A: row sums-of-squares (ACT Square+accum), inv_norm = exp(-0.5 ln ss),
    z = x * inv_norm cast to bf16, transpose via PE matmul-with-identity into a
    [D-part, row] layout, DMA to DRAM, AllGather -> full normalized z^T (bf16).
  - main: sim tile [128 rows, 512 cols] = zT_own.T @ zT_shard accumulated over
    32 K-chunks in PSUM; ACT exp(2*sim) with accum_out produces per-row partial
    denominators; DVE eye-mask extracts the self-sim diagonal and the positive
    pair diagonal from PSUM (valid only on the core's own shard; host selects).
  - KD terms (softmax-free form): kd_i = <e_t, t-s>/Zt + ln Zs - ln Zt with
    e_t = exp(t), Z = sum exp.  MSE from row norms + positive cosine sims.
  - tiny per-row outputs; host combines in float64.
"""
import sys
sys.path.insert(0, '/opt/trn_rl_repo')
import numpy as np
import concourse.bass as bass
import concourse.mybir as mybir
import concourse.tile as tile
from concourse.bass_utils import run_bass_kernel_spmd
from concourse.masks import make_identity

AF = mybir.ActivationFunctionType
FP32 = mybir.dt.float32
BF16 = mybir.dt.bfloat16

B = 2048
D = 4096
N_CORES = 8
RPC = 2 * B // N_CORES          # 512 local rows per core
M_TILES = RPC // 128            # 4 row tiles (0,1 target; 2,3 surrogate)
KC = D // 128                   # 32 contraction chunks
ZT_COLS = KC * RPC              # 16384 cols of zT_own layout [128, KC*RPC]
# output column layout
NCOL_S, NCOL_D, NCOL_P = 0, 32, 64
NCOL_SS, NCOL_Q, NCOL_ZT, NCOL_ZS = 96, 100, 102, 104
OUT_COLS = 106


def _split_sync_waits(nc, max_waits=1):
    """This walrus build rejects >~2 sem waits per instruction; split extras
    onto NOPs inserted before the instruction on the same engine."""
    for blk in nc.main_func.blocks:
        i = 0
        insts = blk.instructions
        while i < len(insts):
            inst = insts[i]
            si = inst.sync_info
            if si is not None and len(si.on_wait) > max_waits:
                waits = list(si.on_wait)
                extra, keep = waits[:-max_waits], waits[-max_waits:]
                pos = i
                for j in range(0, len(extra), max_waits):
                    nop = mybir.InstNoOp(
                        name=nc.get_next_instruction_name(), ins=[], outs=[])
                    nop.engine = inst.engine
                    nop.sync_info = mybir.SyncInfo(
                        on_wait=extra[j:j + max_waits], on_update=[])
                    nc.register_instruction(nop)
                    blk.instructions.insert(pos, nop)
                    pos += 1
                    i += 1
                si.on_wait = keep
                inst.sync_info = si
            i += 1


def _build():
    nc = bass.Bass(num_devices=N_CORES)
    xt = nc.dram_tensor("xt", [RPC // 2, D], FP32, kind="ExternalInput")
    xs = nc.dram_tensor("xs", [RPC // 2, D], FP32, kind="ExternalInput")
    out = nc.dram_tensor("out", [128, OUT_COLS], FP32, kind="ExternalOutput")

    with tile.TileContext(nc) as tc:
        with tc.tile_pool(name="dram", bufs=1, space="DRAM") as dram, \
             tc.tile_pool(name="const", bufs=1) as constp, \
             tc.tile_pool(name="own", bufs=1) as ownp, \
             tc.tile_pool(name="xb", bufs=3) as xpool, \
             tc.tile_pool(name="zb", bufs=2) as zpool, \
             tc.tile_pool(name="eb", bufs=2) as epool, \
             tc.tile_pool(name="db", bufs=1) as dpool, \
             tc.tile_pool(name="wb", bufs=1) as wpool, \
             tc.tile_pool(name="sq", bufs=1) as sqpool, \
             tc.tile_pool(name="ed", bufs=3) as edpool, \
             tc.tile_pool(name="ex", bufs=2) as expool, \
             tc.tile_pool(name="sm", bufs=4) as smpool, \
             tc.tile_pool(name="sh", bufs=3) as shpool, \
             tc.tile_pool(name="pt", bufs=2, space="PSUM") as psum_t, \
             tc.tile_pool(name="ps", bufs=6, space="PSUM") as psum_s:

            zt_local = dram.tile([128, ZT_COLS], BF16)
            zt_all = dram.tile([128 * N_CORES, ZT_COLS], BF16, addr_space="Shared")

            eye = constp.tile([128, 128], BF16)
            make_identity(nc, eye)
            out_t = constp.tile([128, OUT_COLS], FP32)
            zT_own = ownp.tile([128, ZT_COLS], BF16)

            # ---------------- phase A + KD ----------------
            x_tiles = {}
            e_tiles = {}
            for m in (0, 2, 1, 3):  # target half, its surrogate half, ...
                src = xt if m < 2 else xs
                half = m % 2
                xm = xpool.tile([128, D], FP32, name=f"x{m}", tag="xb")
                nc.sync.dma_start(xm, src[128 * half:128 * (half + 1), :])
                x_tiles[m] = xm

                # sum of squares -> ss col; dummy square output
                sqd = sqpool.tile([128, D], BF16, name=f"sq{m}", tag="sq")
                nc.scalar.activation(sqd, xm, AF.Square,
                                     accum_out=out_t[:, NCOL_SS + m:NCOL_SS + m + 1])
                # inv = exp(-0.5 ln ss)
                lns = smpool.tile([128, 1], FP32, name=f"ln{m}", tag="sm")
                nc.scalar.activation(lns, out_t[:, NCOL_SS + m:NCOL_SS + m + 1], AF.Ln)
                inv = smpool.tile([128, 1], FP32, name=f"inv{m}", tag="sm")
                nc.scalar.activation(inv, lns, AF.Exp, scale=-0.5)
                # z = x * inv  (bf16)
                zm = zpool.tile([128, D], BF16, name=f"z{m}", tag="zb")
                nc.vector.tensor_scalar_mul(zm, xm, inv)
                # transpose z into zT_own columns k*512 + 128m
                ztv = zT_own.rearrange("p (k r) -> p k r", r=RPC)
                for kg in range(KC // 4):
                    pt = psum_t.tile([128, 512], FP32, name=f"pt{m}_{kg}", tag="pt")
                    for j in range(4):
                        k = 4 * kg + j
                        nc.tensor.matmul(pt[:, 128 * j:128 * (j + 1)],
                                         zm[:, 128 * k:128 * (k + 1)], eye,
                                         start=True, stop=True)
                    dst = ztv[:, 4 * kg:4 * kg + 4, 128 * m:128 * (m + 1)]
                    nc.scalar.copy(dst, pt.rearrange("p (k r) -> p k r", r=128))

                if m < 2:  # e_t = exp(t), Zt
                    em = epool.tile([128, D], BF16, name=f"e{m}", tag="eb")
                    nc.scalar.activation(em, xm, AF.Exp,
                                         accum_out=out_t[:, NCOL_ZT + m:NCOL_ZT + m + 1])
                    e_tiles[m] = em
                else:      # Zs only
                    sqd2 = sqpool.tile([128, D], BF16, name=f"es{m}", tag="sq")
                    nc.scalar.activation(sqd2, xm, AF.Exp,
                                         accum_out=out_t[:, NCOL_ZS + m - 2:NCOL_ZS + m - 1])
                    # pair (p = m-2): d = t - s ; q = sum(e_t * d)
                    p = m - 2
                    dm = dpool.tile([128, D], BF16, name=f"d{p}", tag="db")
                    nc.vector.tensor_sub(dm, x_tiles[p], xm)
                    wm = wpool.tile([128, D], BF16, name=f"w{p}", tag="wb")
                    nc.vector.tensor_mul(wm, e_tiles[p], dm)
                    nc.vector.reduce_sum(out_t[:, NCOL_Q + p:NCOL_Q + p + 1], wm,
                                         axis=mybir.AxisListType.X)

            nc.sync.dma_start(zt_local, zT_own)
            nc.gpsimd.collective_compute(
                "AllGather", mybir.AluOpType.bypass,
                replica_groups=[list(range(N_CORES))],
                ins=[zt_local.opt()], outs=[zt_all.opt()])

            # ---------------- main sim loop ----------------
            QC = 4                    # quarter-shards streamed per g
            QCOLS = ZT_COLS // QC     # 4096 cols = 1 MiB DMA
            KPQ = KC // QC            # 8 contraction chunks per quarter
            for g in range(N_CORES):
                ps = [psum_s.tile([128, 512], FP32, name=f"ps{g}_{m}", tag="ps")
                      for m in range(M_TILES)]
                for h in range(QC):
                    t = shpool.tile([128, QCOLS], BF16, name=f"sh{g}_{h}", tag="sh")
                    nc.sync.dma_start(
                        t, zt_all[128 * g:128 * (g + 1), QCOLS * h:QCOLS * (h + 1)])
                    for m in range(M_TILES):
                        for kk in range(KPQ):
                            k = KPQ * h + kk
                            nc.tensor.matmul(
                                ps[m],
                                zT_own[:, 512 * k + 128 * m:512 * k + 128 * (m + 1)],
                                t[:, 512 * kk:512 * (kk + 1)],
                                start=(k == 0), stop=(k == KC - 1))
                for m in range(M_TILES):
                    col = 4 * g + m
                    ed = edpool.tile([128, 512], BF16, name=f"ed{g}_{m}", tag="ed")
                    nc.scalar.activation(ed, ps[m], AF.Exp, scale=2.0,
                                         accum_out=out_t[:, NCOL_S + col:NCOL_S + col + 1])
                    # diagonal (self-sim) and positive-pair extraction
                    for base, mm in ((NCOL_D, m), (NCOL_P, (m + 2) % 4)):
                        ext = expool.tile([128, 128], FP32, name=f"ex{g}_{m}_{base}", tag="ex")
                        nc.vector.tensor_mul(ext, ps[m][:, 128 * mm:128 * (mm + 1)], eye)
                        nc.vector.reduce_sum(out_t[:, base + col:base + col + 1], ext,
                                             axis=mybir.AxisListType.X)

            nc.sync.dma_start(out[:], out_t)

    _split_sync_waits(nc)
    return nc


_NC_CACHE = None


def _get_nc():
    global _NC_CACHE
    if _NC_CACHE is None:
        _NC_CACHE = _build()
    return _NC_CACHE


def run_device(emb_target, emb_surrogate, trace=False, **kw):
    emb_target = np.ascontiguousarray(emb_target, dtype=np.float32)
    emb_surrogate = np.ascontiguousarray(emb_surrogate, dtype=np.float32)
    nc = _get_nc()
    rc = RPC // 2
    in_maps = [
        {"xt": emb_target[rc * c:rc * (c + 1)], "xs": emb_surrogate[rc * c:rc * (c + 1)]}
        for c in range(N_CORES)
    ]
    return run_bass_kernel_spmd(nc, in_maps, core_ids=list(range(N_CORES)),
                                trace=trace, **kw)


def combine(results):
    total_cont = 0.0
    total_kd = 0.0
    total_mse = 0.0
    for c in range(N_CORES):
        o = results[c]["out"].astype(np.float64)
        S = o[:, NCOL_S:NCOL_S + 32].reshape(128, 8, 4).sum(axis=1)
        diag = o[:, NCOL_D:NCOL_D + 32].reshape(128, 8, 4)[:, c, :]
        pos = o[:, NCOL_P:NCOL_P + 32].reshape(128, 8, 4)[:, c, :]
        ss = o[:, NCOL_SS:NCOL_SS + 4]
        q = o[:, NCOL_Q:NCOL_Q + 2]
        Zt = o[:, NCOL_ZT:NCOL_ZT + 2]
        Zs = o[:, NCOL_ZS:NCOL_ZS + 2]
        denom = S - np.exp(2.0 * diag)
        total_cont += (np.log(denom) - 2.0 * pos).sum()
        total_kd += (q / Zt + np.log(Zs) - np.log(Zt)).sum()
        nt = np.sqrt(ss[:, 0:2])
        ns = np.sqrt(ss[:, 2:4])
        total_mse += (ss[:, 0:2] + ss[:, 2:4] - 2.0 * pos[:, 0:2] * nt * ns).sum()
    loss = (total_cont / (2 * B)) + (total_kd / B) + (total_mse / (B * D))
    return np.float32(loss)


def kernel(emb_target, emb_surrogate):
    res = run_device(emb_target, emb_surrogate)
    return combine(res.results)


if __name__ == "__main__":
    rng = np.random.default_rng(0)
    t = rng.standard_normal((B, D)).astype(np.float32)
    s = rng.standard_normal((B, D)).astype(np.float32)
    print(kernel(t, s))


# revision 18
# speedup vs baseline: 3.1218x; 3.1218x over previous
"""Trainium2 Bass kernel for nn_ContKDLoss (NT-Xent contrastive + KD softmax-KL + MSE).

Strategy (8 NeuronCores, data parallel over batch):
  - core c owns 256 target rows + 256 surrogate rows (512 "local rows").
  - phase A: row sums-of-squares (ACT Square+accum), inv_norm = exp(-0.5 ln ss),
    z = x * inv_norm cast to bf16, transpose via PE matmul-with-identity into a
    [D-part, row] layout, DMA to DRAM, AllGather -> full normalized z^T (bf16).
  - main: sim tile [128 rows, 512 cols] = zT_own.T @ zT_shard accumulated over
    32 K-chunks in PSUM; ACT exp(2*sim) with accum_out produces per-row partial
    denominators; DVE eye-mask extracts the self-sim diagonal and the positive
    pair diagonal from PSUM (valid only on the core's own shard; host selects).
  - KD terms (softmax-free form): kd_i = <e_t, t-s>/Zt + ln Zs - ln Zt with
    e_t = exp(t), Z = sum exp.  MSE from row norms + positive cosine sims.
  - tiny per-row outputs; host combines in float64.
"""
import os
import sys
sys.path.insert(0, '/opt/trn_rl_repo')
import numpy as np
import concourse.bass as bass
import concourse.mybir as mybir
import concourse.tile as tile
from concourse.bass_utils import run_bass_kernel_spmd
from concourse.masks import make_identity

AF = mybir.ActivationFunctionType
FP32 = mybir.dt.float32
BF16 = mybir.dt.bfloat16
FP8 = mybir.dt.float8e4

B = 2048
D = 4096
N_CORES = 8
RPC = 2 * B // N_CORES          # 512 local rows per core
M_TILES = RPC // 128            # 4 row tiles (0,1 target; 2,3 surrogate)
KC = D // 128                   # 32 contraction chunks
ZT_COLS = KC * RPC              # 16384 cols of zT_own layout [128, KC*RPC]
# output column layout
NCOL_S, NCOL_D, NCOL_P = 0, 32, 64
NCOL_SS, NCOL_Q, NCOL_ZT, NCOL_ZS = 96, 100, 102, 104
OUT_COLS = 106


def _split_sync_waits(nc, max_waits=1):
    """This walrus build rejects >~2 sem waits per instruction; split extras
    onto NOPs inserted before the instruction on the same engine."""
    for blk in nc.main_func.blocks:
        i = 0
        insts = blk.instructions
        while i < len(insts):
            inst = insts[i]
            si = inst.sync_info
            if si is not None and len(si.on_wait) > max_waits:
                waits = list(si.on_wait)
                extra, keep = waits[:-max_waits], waits[-max_waits:]
                pos = i
                for j in range(0, len(extra), max_waits):
                    nop = mybir.InstNoOp(
                        name=nc.get_next_instruction_name(), ins=[], outs=[])
                    nop.engine = inst.engine
                    nop.sync_info = mybir.SyncInfo(
                        on_wait=extra[j:j + max_waits], on_update=[])
                    nc.register_instruction(nop)
                    blk.instructions.insert(pos, nop)
                    pos += 1
                    i += 1
                si.on_wait = keep
                inst.sync_info = si
            i += 1


def _build():
    nc = bass.Bass(num_devices=N_CORES)
    xt = nc.dram_tensor("xt", [RPC // 2, D], FP32, kind="ExternalInput")
    xs = nc.dram_tensor("xs", [RPC // 2, D], FP32, kind="ExternalInput")
    out = nc.dram_tensor("out", [128, OUT_COLS], FP32, kind="ExternalOutput")

    with tile.TileContext(nc) as tc:
        with tc.tile_pool(name="dram", bufs=1, space="DRAM") as dram, \
             tc.tile_pool(name="const", bufs=1) as constp, \
             tc.tile_pool(name="own", bufs=1) as ownp, \
             tc.tile_pool(name="xb", bufs=4) as xpool, \
             tc.tile_pool(name="zb", bufs=2) as zpool, \
             tc.tile_pool(name="eb", bufs=2) as epool, \
             tc.tile_pool(name="db", bufs=1) as dpool, \
             tc.tile_pool(name="wb", bufs=1) as wpool, \
             tc.tile_pool(name="sq", bufs=1) as sqpool, \
             tc.tile_pool(name="ed", bufs=3) as edpool, \
             tc.tile_pool(name="ex", bufs=2) as expool, \
             tc.tile_pool(name="sm", bufs=4) as smpool, \
             tc.tile_pool(name="sh", bufs=3) as shpool, \
             tc.tile_pool(name="pt", bufs=2, space="PSUM") as psum_t, \
             tc.tile_pool(name="ps", bufs=6, space="PSUM") as psum_s:

            zt_local = dram.tile([128, ZT_COLS], FP8)
            zt_all = dram.tile([128 * N_CORES, ZT_COLS], FP8, addr_space="Shared")

            eye = constp.tile([128, 128], FP8)
            make_identity(nc, eye)
            eye_ex = constp.tile([128, 128], BF16)
            make_identity(nc, eye_ex)
            out_t = constp.tile([128, OUT_COLS], FP32)
            nc.gpsimd.memset(out_t, 0.0)
            zT_own = ownp.tile([128, ZT_COLS], FP8)

            # ---------------- phase A + KD ----------------
            x_tiles = {}
            e_tiles = {}
            for m in (0, 2, 1, 3):  # target half, its surrogate half, ...
                src = xt if m < 2 else xs
                half = m % 2
                xm = xpool.tile([128, D], FP32, name=f"x{m}", tag="xb")
                nc.sync.dma_start(xm, src[128 * half:128 * (half + 1), :])
                x_tiles[m] = xm

                # sum of squares -> ss col; dummy square output
                sqd = sqpool.tile([128, D], BF16, name=f"sq{m}", tag="sq")
                nc.scalar.activation(sqd, xm, AF.Square,
                                     accum_out=out_t[:, NCOL_SS + m:NCOL_SS + m + 1])
                # inv = exp(-0.5 ln ss)
                lns = smpool.tile([128, 1], FP32, name=f"ln{m}", tag="sm")
                nc.scalar.activation(lns, out_t[:, NCOL_SS + m:NCOL_SS + m + 1], AF.Ln)
                inv = smpool.tile([128, 1], FP32, name=f"inv{m}", tag="sm")
                nc.scalar.activation(inv, lns, AF.Exp, scale=-0.5)
                # z = x * inv  (bf16)
                zm = zpool.tile([128, D], FP8, name=f"z{m}", tag="zb")
                nc.vector.tensor_scalar_mul(zm, xm, inv)
                # transpose z into zT_own columns k*512 + 128m
                ztv = zT_own.rearrange("p (k r) -> p k r", r=RPC)
                for kg in range(KC // 4):
                    pt = psum_t.tile([128, 512], FP32, name=f"pt{m}_{kg}", tag="pt")
                    for j in range(4):
                        k = 4 * kg + j
                        nc.tensor.matmul(pt[:, 128 * j:128 * (j + 1)],
                                         zm[:, 128 * k:128 * (k + 1)], eye,
                                         start=True, stop=True)
                    dst = ztv[:, 4 * kg:4 * kg + 4, 128 * m:128 * (m + 1)]
                    src = pt.rearrange("p (k r) -> p k r", r=128)
                    if kg % 2 == 0:
                        nc.scalar.copy(dst, src)
                    else:
                        nc.vector.tensor_copy(dst, src)

                if m < 2:  # e_t = exp(t), Zt
                    em = epool.tile([128, D], BF16, name=f"e{m}", tag="eb")
                    nc.scalar.activation(em, xm, AF.Exp,
                                         accum_out=out_t[:, NCOL_ZT + m:NCOL_ZT + m + 1])
                    e_tiles[m] = em
                else:      # Zs only
                    sqd2 = sqpool.tile([128, D], BF16, name=f"es{m}", tag="sq")
                    nc.scalar.activation(sqd2, xm, AF.Exp,
                                         accum_out=out_t[:, NCOL_ZS + m - 2:NCOL_ZS + m - 1])
                    # pair (p = m-2): d = t - s ; q = sum(e_t * d)
                    p = m - 2
                    dm = dpool.tile([128, D], BF16, name=f"d{p}", tag="db")
                    nc.vector.tensor_sub(dm, x_tiles[p], xm)
                    wm = wpool.tile([128, D], BF16, name=f"w{p}", tag="wb")
                    nc.vector.tensor_mul(wm, e_tiles[p], dm)
                    nc.vector.reduce_sum(out_t[:, NCOL_Q + p:NCOL_Q + p + 1], wm,
                                         axis=mybir.AxisListType.X)

            nc.sync.dma_start(zt_local, zT_own)
            nc.gpsimd.collective_compute(
                "AllGather", mybir.AluOpType.bypass,
                replica_groups=[list(range(N_CORES))],
                ins=[zt_local.opt()], outs=[zt_all.opt()])

            # ---------------- main sim loop ----------------
            QC = 2                    # half-shards streamed per g (fp8: 1 MiB DMA)
            QCOLS = ZT_COLS // QC
            KPQ = KC // QC
            for g in range(int(os.environ.get("KMAIN_G", str(N_CORES)))):
                ps = [psum_s.tile([128, 512], FP32, name=f"ps{g}_{m}", tag="ps")
                      for m in range(M_TILES)]
                ztv512 = zT_own.rearrange("p (k r) -> p k r", r=RPC)
                for h in range(QC):
                    t = shpool.tile([128, QCOLS], FP8, name=f"sh{g}_{h}", tag="sh")
                    nc.sync.dma_start(
                        t, zt_all[128 * g:128 * (g + 1), QCOLS * h:QCOLS * (h + 1)])
                    tv = t.rearrange("p (k r) -> p k r", r=RPC)
                    for m in range(M_TILES):
                        for tt in range(KPQ // 2):
                            k = KPQ * h + 2 * tt
                            nc.tensor.matmul(
                                ps[m],
                                ztv512[:, k:k + 2, 128 * m:128 * (m + 1)],
                                tv[:, 2 * tt:2 * tt + 2, :],
                                start=(k == 0), stop=(k == KC - 2),
                                perf_mode=mybir.MatmulPerfMode.DoubleRow)
                for m in range(M_TILES):
                    col = 4 * g + m
                    ed = edpool.tile([128, 512], BF16, name=f"ed{g}_{m}", tag="ed")
                    nc.scalar.activation(ed, ps[m], AF.Exp, scale=2.0,
                                         accum_out=out_t[:, NCOL_S + col:NCOL_S + col + 1])
                    # diagonal (self-sim) and positive-pair extraction
                    for base, mm in ((NCOL_D, m), (NCOL_P, (m + 2) % 4)):
                        ext = expool.tile([128, 128], FP32, name=f"ex{g}_{m}_{base}", tag="ex")
                        nc.vector.tensor_mul(ext, ps[m][:, 128 * mm:128 * (mm + 1)], eye_ex)
                        nc.vector.reduce_sum(out_t[:, base + col:base + col + 1], ext,
                                             axis=mybir.AxisListType.X)

            nc.sync.dma_start(out[:], out_t)

    _split_sync_waits(nc)
    return nc


_NC_CACHE = None


def _get_nc():
    global _NC_CACHE
    if _NC_CACHE is None:
        _NC_CACHE = _build()
    return _NC_CACHE


def run_device(emb_target, emb_surrogate, trace=False, **kw):
    emb_target = np.ascontiguousarray(emb_target, dtype=np.float32)
    emb_surrogate = np.ascontiguousarray(emb_surrogate, dtype=np.float32)
    nc = _get_nc()
    rc = RPC // 2
    in_maps = [
        {"xt": emb_target[rc * c:rc * (c + 1)],
         "xs": emb_surrogate[rc * c:rc * (c + 1)]}
        for c in range(N_CORES)
    ]
    return run_bass_kernel_spmd(nc, in_maps, core_ids=list(range(N_CORES)),
                                trace=trace, **kw)


def combine(results):
    total_cont = 0.0
    total_kd = 0.0
    total_mse = 0.0
    for c in range(N_CORES):
        o = results[c]["out"].astype(np.float64)
        S = o[:, NCOL_S:NCOL_S + 32].reshape(128, 8, 4).sum(axis=1)
        diag = o[:, NCOL_D:NCOL_D + 32].reshape(128, 8, 4)[:, c, :]
        pos = o[:, NCOL_P:NCOL_P + 32].reshape(128, 8, 4)[:, c, :]
        ss = o[:, NCOL_SS:NCOL_SS + 4]
        q = o[:, NCOL_Q:NCOL_Q + 2]
        Zt = o[:, NCOL_ZT:NCOL_ZT + 2]
        Zs = o[:, NCOL_ZS:NCOL_ZS + 2]
        denom = S - np.exp(2.0 * diag)
        total_cont += (np.log(denom) - 2.0 * pos).sum()
        total_kd += (q / Zt + np.log(Zs) - np.log(Zt)).sum()
        nt = np.sqrt(ss[:, 0:2])
        ns = np.sqrt(ss[:, 2:4])
        total_mse += (ss[:, 0:2] + ss[:, 2:4] - 2.0 * pos[:, 0:2] * nt * ns).sum()
    loss = (total_cont / (2 * B)) + (total_kd / B) + (total_mse / (B * D))
    return np.float32(loss)


def kernel(emb_target, emb_surrogate):
    res = run_device(emb_target, emb_surrogate)
    return combine(res.results)


if __name__ == "__main__":
    rng = np.random.default_rng(0)
    t = rng.standard_normal((B, D)).astype(np.float32)
    s = rng.standard_normal((B, D)).astype(np.float32)
    print(kernel(t, s))
